# revision 16
# baseline (speedup 1.0000x reference)
"""Trainium2 Bass kernel for the 16-head masked-attention module.

Math per head (reference):
    q = Q @ Wq.T + bq ; k = K @ Wk.T + bk ; v = V @ Wv.T + bv      [S, 64]
    qk = tril(q @ k.T)                 (zeroed, not -inf)
    scores = log_softmax(qk / 8, axis=0)   (softmax over the QUERY axis,
                                            per key column)
    attn = scores @ v
    out = concat(heads) @ WO.T + bO

Device decomposition (8 cores, 2 heads/core, tensor-parallel over heads,
WO row-sharded; host sums the 8 partial outputs, transposes, adds bO):

    scores[s,t] = msc[t,s]/8 - lse[t]   where msc = masked raw qk (transposed
    layout, [t,s]), lse[t] = log(t + sum_{s>=t} exp(msc[t,s]/8))  (the t
    masked zeros contribute exp(0)=1 each).

    attn[s,:] = (1/8)*sum_t msc[t,s] v[t,:]  -  sum_t lse[t] v[t,:]
              = (1/8)*(prefix part + diagonal part) - corr
    with the fully-kept 128-blocks collapsed via rank-64 prefix sums:
        C_m = k_m^T v_m  [64,64];  P_m = sum_{m'<m} C_m'
        prefix part for s-chunk m = (q_m @ P_m)
    Only the 16 diagonal 128x128 triangles are materialized in SBUF.

    The output is computed TRANSPOSED on-device (outT[d, s]) so the
    lse-dependent correction (wcorr[d] = corr @ WO.T) is a per-partition
    scalar applied late; this lets the attn assembly and the WO matmuls run
    during the exp grind instead of after it. Host sums the 8 partials,
    transposes, and adds bO.
"""

import numpy as np

S = 2048
D = 1024
NCORES = 8

_CACHE = {}


def _split_multi_waits(nc, mybir, max_waits=1):
    """This walrus build only encodes one sync-wait per instruction; Tile's
    tail drain carries several. Hoist extras onto preceding NoOps."""
    n = 0
    for fn in nc.m.functions:
        for blk in fn.blocks:
            out = []
            changed = False
            for ins in blk.instructions:
                si = getattr(ins, "sync_info", None)
                waits = list(si.on_wait) if (si is not None and si.on_wait) else []
                if len(waits) > max_waits:
                    for w in waits[:-max_waits]:
                        nop = mybir.InstNoOp(
                            name=nc.get_next_instruction_name(), ins=[], outs=[]
                        )
                        nop.engine = ins.engine
                        nop.sync_info = mybir.SyncInfo(on_wait=[w], on_update=[])
                        out.append(nop)
                        n += 1
                    si.on_wait = waits[-max_waits:]
                    changed = True
                out.append(ins)
            if changed:
                blk.instructions = out
    return n


def _build(loop_n=1):
    import concourse.bass as bass
    import concourse.mybir as mybir
    import concourse.tile as tile
    from concourse.bass import ts
    from concourse.masks import make_identity

    F32 = mybir.dt.float32
    BF16 = mybir.dt.bfloat16
    FP8 = mybir.dt.float8e4
    AF = mybir.ActivationFunctionType
    OP = mybir.AluOpType

    nc = bass.Bass("TRN2", num_devices=NCORES, debug=False)

    qt_d = nc.dram_tensor("qt", [D, S], FP8, kind="ExternalInput")
    kt_d = nc.dram_tensor("kt", [D, S], FP8, kind="ExternalInput")
    vt_d = nc.dram_tensor("vt", [D, S], BF16, kind="ExternalInput")
    # packed constants: wbf1 = [wq | wk] (qk-side weights), wbf2 = [wv | wo],
    # cf = [bq | bk | bv-row | tm | ct] (f32)
    wbf1_d = nc.dram_tensor("wbf1", [128, 2048], BF16, kind="ExternalInput")
    wbf2_d = nc.dram_tensor("wbf2", [128, 2048], BF16, kind="ExternalInput")
    cf_d = nc.dram_tensor("cf", [128, 658], F32, kind="ExternalInput")
    out_d = nc.dram_tensor("out", [D, S], BF16, kind="ExternalOutput")
    wcorr_d = nc.dram_tensor("wcorr", [128, 8], F32, kind="ExternalOutput")

    with tile.TileContext(nc) as tc:
        with (
            tc.tile_pool(name="singles", bufs=1) as sg,
            tc.tile_pool(name="instream", bufs=6) as instream,
            tc.tile_pool(name="scratch", bufs=2) as scratch,
            tc.tile_pool(name="outs", bufs=1) as outs,
        ):
            # ---- constants (issued on the scalar HWDGE ring so the input
            # streams own the sync ring) ----
            wbf1 = sg.tile([128, 2048], BF16, tag="wbf1")
            wbf2 = sg.tile([128, 2048], BF16, tag="wbf2")
            cf = sg.tile([128, 658], F32, tag="cf")
            ident = sg.tile([128, 128], BF16, tag="ident")
            ones_r = sg.tile([1, 128], BF16, tag="ones_r")
            bv_bf = sg.tile([1, 512], BF16, tag="bv_bf")
            nc.scalar.dma_start(wbf1[:], wbf1_d[:])
            nc.scalar.dma_start(cf[:], cf_d[:])
            nc.scalar.dma_start(wbf2[:], wbf2_d[:])
            wq_sb = wbf1[:, 0:1024].rearrange("p (o f) -> p o f", f=128)
            wk_sb = wbf1[:, 1024:2048].rearrange("p (o f) -> p o f", f=128)
            wv_sb = wbf2[:, 0:1024].rearrange("p (o f) -> p o f", f=128)
            wo_sb = wbf2[:, 1024:2048]
            bq_sb = cf[:, 0:1]
            bk_sb = cf[:, 1:2]
            bv_row = cf[0:1, 2:514]   # bv tiled x4 for grouped V psum
            tm_sb = cf[:, 514:642]
            ct_sb = cf[:, 642:658]
            make_identity(nc, ident[:])
            nc.vector.memset(ones_r[:], 1.0)
            nc.vector.tensor_copy(bv_bf[:], bv_row)

            # ---- persistent activations ----
            qT = sg.tile([128, S], BF16, tag="qT")   # [dk(2 heads), s]
            kT = sg.tile([128, S], BF16, tag="kT")
            vt_sb = sg.tile([128, 8, S], BF16, tag="vt_sb")   # raw V input
            k_sb = sg.tile([128, 16, 128], BF16, tag="k_sb")  # [t, chunk, dk]
            v_sb = sg.tile([128, 2048], BF16, tag="v_sb")     # [t, chunk*dv]
            p_bf = sg.tile([128, 16, 64], BF16, tag="p_bf")   # prefix sums
            zA = sg.tile([128, S], BF16, tag="zA")            # attn/8, no corr
            sums_a = [sg.tile([128, 16], F32, tag=f"s_a{h}", name=f"s_a{h}")
                      for h in range(2)]
            sums_b = [sg.tile([128, 16], F32, tag=f"s_b{h}", name=f"s_b{h}")
                      for h in range(2)]
            lse_f = [sg.tile([128, 16], F32, tag=f"lse_f{h}", name=f"lse_f{h}")
                     for h in range(2)]
            lse_b = [sg.tile([128, 16], BF16, tag=f"lse_b{h}", name=f"lse_b{h}")
                     for h in range(2)]
            corr_f = sg.tile([128, 1], F32, tag="corr_f")
            corr_b = sg.tile([128, 1], BF16, tag="corr_b")
            o_sbT = sg.tile([128, 8, S], BF16, tag="o_sbT")
            tri = [
                [sg.tile([128, 128], BF16, tag=f"tri{h}_{i}", name=f"tri{h}_{i}")
                 for i in range(16)]
                for h in range(2)
            ]

            def emit_body():
                _emit_phases(
                    nc, tc, tile, mybir, ts, F32, BF16, FP8, AF, OP,
                    qt_d, kt_d, vt_d, out_d, wcorr_d,
                    wq_sb, wk_sb, wv_sb, wo_sb, bq_sb, bk_sb, bv_bf, ones_r,
                    tm_sb, ct_sb, ident, instream, scratch, outs,
                    qT, kT, vt_sb, k_sb, v_sb, p_bf, zA,
                    sums_a, sums_b, lse_f, lse_b, corr_f, corr_b,
                    o_sbT, tri,
                )

            if loop_n == 1:
                emit_body()
            else:
                with tc.For_i(0, loop_n, 1):
                    emit_body()

    _split_multi_waits(nc, mybir)
    return nc


def _emit_phases(
    nc, tc, tile, mybir, ts, F32, BF16, FP8, AF, OP,
    qt_d, kt_d, vt_d, out_d, wcorr_d,
    wq_sb, wk_sb, wv_sb, wo_sb, bq_sb, bk_sb, bv_bf, ones_r,
    tm_sb, ct_sb, ident, instream, scratch, outs,
    qT, kT, vt_sb, k_sb, v_sb, p_bf, zA,
    sums_a, sums_b, lse_f, lse_b, corr_f, corr_b,
    o_sbT, tri,
):
    from contextlib import ExitStack

    for h in range(2):
        nc.vector.memset(sums_b[h][:], 0.0)

    # ---- Phase 1: K and Q projections -> kT/qT [dk, s]; K transposes to
    # k_sb [t, dk] fill PE gaps in the DMA-bound Q window. ----
    with tc.tile_pool(name="pp", bufs=4, space="PSUM") as pp, \
         tc.tile_pool(name="pt", bufs=2, space="PSUM") as pt:
        for name, src_d, w_sb, b_sb, dstT in (
            ("k", kt_d, wk_sb, bk_sb, kT),
            ("q", qt_d, wq_sb, bq_sb, qT),
        ):
            ps = [pp.tile([128, 512], F32, tag="pp", name=f"pp_{name}{j}")
                  for j in range(4)]
            for o in range(8):
                chunk = instream.tile([128, S], FP8, tag="in")
                nc.sync.dma_start(chunk[:], src_d[ts(o, 128), :])
                if name == "q" and o % 2 == 1:
                    # V input blocks share the sync ring behind the q chunks
                    gg = (o - 1) // 2
                    nc.sync.dma_start(
                        vt_sb[:, :, ts(gg, 512)],
                        vt_d[:, ts(gg, 512)].rearrange(
                            "(o2 p) c -> p o2 c", p=128),
                    )
                for j in range(4):
                    nc.tensor.matmul(
                        ps[j][:], w_sb[:, o, :], chunk[:, ts(j, 512)],
                        start=(o == 0), stop=(o == 7),
                    )
                if name == "q":
                    # K transposes interleave with the DMA-gated Q matmuls;
                    # evacuations go to ACT, which is idle in this window
                    for m in (2 * o, 2 * o + 1):
                        ptile = pt.tile([128, 128], BF16, tag="pt",
                                        name=f"pt_{m}")
                        nc.tensor.transpose(ptile[:], kT[:, ts(m, 128)],
                                            ident[:])
                        nc.vector.tensor_copy(k_sb[:, m, :], ptile[:])
            for j in range(4):
                nc.scalar.activation(
                    dstT[:, ts(j, 512)], ps[j][:], AF.Identity,
                    bias=b_sb[:], scale=1.0,
                )

    # ---- Phase 2: score rows [t,s] + exp sums, with V projection, prefix
    # sums, and attn assembly slotted between rows so the PE stream stays
    # dense (HAM stays warm) while ACT grinds the exp sweep. ----
    def emit_row(i, pqA, pqB):
        w = S - 128 * i
        wa = min(w, 1536)
        for h in range(2):
            hp = slice(64 * h, 64 * h + 64)
            pa = pqA.tile([128, 1536], F32, tag="pqA", name=f"pa_{h}_{i}")
            for c0 in range(0, wa, 512):
                cw = min(512, wa - c0)
                nc.tensor.matmul(
                    pa[:, c0:c0 + cw],
                    kT[hp, ts(i, 128)],
                    qT[hp, 128 * i + c0:128 * i + c0 + cw],
                    start=True, stop=True,
                )
            # masked diag triangle -> bf16 SBUF, write back so the exp
            # sweep sees masked values
            nc.vector.tensor_tensor(
                tri[h][i][:], pa[:, 0:128], tm_sb[:], OP.mult
            )
            nc.vector.tensor_copy(pa[:, 0:128], tri[h][i][:])
            ea = scratch.tile([128, 1536], BF16, tag="ea", name=f"ea_{h}_{i}")
            nc.scalar.activation(
                ea[:, :wa], pa[:, :wa], AF.Exp,
                scale=0.125, accum_out=sums_a[h][:, i:i + 1],
            )
            if w > 1536:
                bw = w - 1536
                pb = pqB.tile([128, 512], F32, tag="pqB", name=f"pb_{h}_{i}")
                nc.tensor.matmul(
                    pb[:, 0:bw],
                    kT[hp, ts(i, 128)],
                    qT[hp, 128 * i + 1536:2048],
                    start=True, stop=True,
                )
                eb = scratch.tile([128, 512], BF16, tag="eb",
                                  name=f"eb_{h}_{i}")
                nc.scalar.activation(
                    eb[:, :bw], pb[:, :bw], AF.Exp,
                    scale=0.125, accum_out=sums_b[h][:, i:i + 1],
                )

    with tc.tile_pool(name="pqA", bufs=2, space="PSUM") as pqA, \
         tc.tile_pool(name="paux", bufs=1, space="PSUM") as paux, \
         tc.tile_pool(name="pshare", bufs=1, space="PSUM") as pshare:
        # paux's single bank carries the B-tiles of rows 0-3, then becomes
        # the prefix-sum accumulator; pshare's bank alternates V-projection
        # groups and attn-assembly tiles.
        prefix_done = 0
        ctile = None
        pvt = None
        for i in range(16):
            g, mm = i // 4, i % 4
            emit_row(i, pqA, paux if i < 4 else None)
            # V projection chunk i interleaves after each row so the PE
            # instruction stream never head-of-line blocks on exp psum
            if mm == 0:
                pvt = pshare.tile([128, 512], F32, tag="sh", name=f"pv_{g}")
                nc.tensor.matmul(pvt[:], ones_r[:], bv_bf[:],
                                 start=True, stop=False,
                                 skip_group_check=True)
            for o in range(8):
                nc.tensor.matmul(
                    pvt[:, ts(mm, 128)],
                    vt_sb[:, o, ts(i, 128)], wv_sb[:, o, :],
                    start=False, stop=(mm == 3 and o == 7),
                    skip_group_check=True,
                )
            if mm != 3:
                continue
            # ---- end of slot g ----
            nc.vector.tensor_copy(v_sb[:, ts(g, 512)], pvt[:])
            if g == 0:
                nc.vector.memset(p_bf[:, 0, :], 0.0)
                ctile = paux.tile([128, 512], F32, tag="pqB", name="ctile")
            # prefix sums P_m: running psum accumulation with snapshot
            # evacuations; attn group g needs P up to m=4g+3
            while prefix_done <= min(4 * g + 2, 14):
                m = prefix_done
                nc.tensor.matmul(
                    ctile[0:64, 0:64], k_sb[:, m, 0:64],
                    v_sb[:, 128 * m:128 * m + 64],
                    start=(m == 0), stop=(m == 14), skip_group_check=True,
                )
                nc.tensor.matmul(
                    ctile[64:128, 0:64], k_sb[:, m, 64:128],
                    v_sb[:, 128 * m + 64:128 * m + 128],
                    start=(m == 0), stop=(m == 14), skip_group_check=True,
                    tile_position=(0, 64),
                )
                nc.vector.tensor_copy(p_bf[:, m + 1, :], ctile[:, 0:64])
                prefix_done += 1
            # attn assembly: patt[hv, s] = prefix part + diag part, then
            # zA = patt/8 (corr is applied host-side via wcorr)
            patt = pshare.tile([128, 512], F32, tag="sh", name=f"pat_{g}")
            for mm2 in range(4):
                m = 4 * g + mm2
                cols = ts(mm2, 128)
                if m > 0:
                    nc.tensor.matmul(
                        patt[0:64, cols], p_bf[0:64, m, :],
                        qT[0:64, ts(m, 128)], start=True, stop=False,
                    )
                    nc.tensor.matmul(
                        patt[64:128, cols], p_bf[64:128, m, :],
                        qT[64:128, ts(m, 128)],
                        start=True, stop=False, tile_position=(64, 64),
                    )
                nc.tensor.matmul(
                    patt[0:64, cols], v_sb[:, 128 * m:128 * m + 64],
                    tri[0][m][:],
                    start=(m == 0), stop=True,
                )
                nc.tensor.matmul(
                    patt[64:128, cols], v_sb[:, 128 * m + 64:128 * m + 128],
                    tri[1][m][:],
                    start=(m == 0), stop=True, tile_position=(0, 64),
                )
            nc.vector.tensor_scalar(
                zA[:, ts(g, 512)], patt[:], 0.125, None, OP.mult,
            )
            # WO matmuls for this group (transposed output), sharing the
            # pshare bank; evacuations on DVE
            for j in range(8):
                pot = pshare.tile([128, 512], F32, tag="sh",
                                  name=f"po_{g}_{j}")
                nc.tensor.matmul(
                    pot[:], wo_sb[:, ts(j, 128)], zA[:, ts(g, 512)],
                    start=True, stop=True,
                )
                nc.vector.tensor_copy(o_sbT[:, j, ts(g, 512)], pot[:])
            if g == 3:
                # two 2 MB output batches on the scalar ring (sync ring is
                # owned by the next iteration's input streams)
                for b in range(2):
                    nc.scalar.dma_start(
                        out_d[ts(b, 512), :].rearrange(
                            "(c p) s -> p c s", p=128),
                        o_sbT[:, 4 * b:4 * b + 4, :],
                    )

    # ---- Phase 4: lse, corr, wcorr (shipped; host applies the rank-1
    # correction during unshard) ----
    with tc.tile_pool(name="pcr", bufs=1, space="PSUM") as pcr:
        for h in range(2):
            nc.vector.tensor_tensor(
                lse_f[h][:], sums_a[h][:], sums_b[h][:], OP.add
            )
            nc.vector.tensor_tensor(lse_f[h][:], lse_f[h][:], ct_sb[:], OP.add)
            nc.scalar.activation(lse_f[h][:], lse_f[h][:], AF.Ln, scale=1.0)
            nc.vector.tensor_copy(lse_b[h][:], lse_f[h][:])
        cps = pcr.tile([128, 1], F32, tag="pcr")
        for i in range(16):
            nc.tensor.matmul(
                cps[0:64, :], v_sb[:, 128 * i:128 * i + 64], lse_b[0][:, i:i + 1],
                start=(i == 0), stop=(i == 15),
            )
            nc.tensor.matmul(
                cps[64:128, :], v_sb[:, 128 * i + 64:128 * i + 128], lse_b[1][:, i:i + 1],
                start=(i == 0), stop=(i == 15), tile_position=(0, 64),
            )
        nc.vector.tensor_copy(corr_f[:], cps[:])
        nc.vector.tensor_copy(corr_b[:], corr_f[:])
        pw = pcr.tile([128, 8], F32, tag="pw")
        for j in range(8):
            nc.tensor.matmul(
                pw[:, j:j + 1], wo_sb[:, ts(j, 128)], corr_b[:],
                start=True, stop=True,
            )
        wc_sb = scratch.tile([128, 8], F32, tag="wc")
        nc.vector.tensor_scalar(wc_sb[:], pw[:], -1.0, None, OP.mult)
        nc.sync.dma_start(wcorr_d[:], wc_sb[:])


def _get_program(loop_n=1):
    key = f"nc{loop_n}"
    if key not in _CACHE:
        _CACHE[key] = _build(loop_n)
    return _CACHE[key]


def _get_exec(loop_n=1):
    """Build the sharded PJRT executable once (same lowering path as
    concourse.bass2jax.run_bass_via_pjrt, hoisted so repeat calls don't
    re-trace/re-compile)."""
    key = f"exec{loop_n}"
    if key in _CACHE:
        return _CACHE[key]
    import jax
    import numpy as _np
    from jax.experimental.shard_map import shard_map
    from jax.sharding import Mesh, PartitionSpec
    import concourse.mybir as mybir
    from concourse import bass2jax

    nc = _get_program(loop_n)
    bass2jax.install_neuronx_cc_hook()

    partition_name = (
        nc.partition_id_tensor.name if nc.partition_id_tensor else None
    )
    in_names, out_names, out_avals = [], [], []
    for alloc in nc.m.functions[0].allocations:
        if not isinstance(alloc, mybir.MemoryLocationSet):
            continue
        name = alloc.memorylocations[0].name
        if alloc.kind == "ExternalInput":
            if name != partition_name:
                in_names.append(name)
        elif alloc.kind == "ExternalOutput":
            out_names.append(name)
            out_avals.append(
                jax.core.ShapedArray(
                    tuple(alloc.tensor_shape), mybir.dt.np(alloc.dtype)
                )
            )
    n_params = len(in_names)
    n_outs = len(out_avals)
    all_names = in_names + out_names
    if partition_name is not None:
        all_names = all_names + [partition_name]

    def _body(*args):
        operands = list(args)
        if partition_name is not None:
            operands.append(bass2jax.partition_id_tensor())
        outs = bass2jax._bass_exec_p.bind(
            *operands,
            out_avals=tuple(out_avals),
            in_names=tuple(all_names),
            out_names=tuple(out_names),
            lowering_input_output_aliases=(),
            sim_require_finite=True,
            sim_require_nnan=True,
            nc=nc,
        )
        return tuple(outs)

    devices = jax.devices()[:NCORES]
    mesh = Mesh(_np.asarray(devices), ("core",))
    donate = tuple(range(n_params, n_params + n_outs))
    sharded = jax.jit(
        shard_map(
            _body,
            mesh=mesh,
            in_specs=(PartitionSpec("core"),) * (n_params + n_outs),
            out_specs=(PartitionSpec("core"),) * n_outs,
            check_rep=False,
        ),
        donate_argnums=donate,
        keep_unused=True,
    )
    _CACHE[key] = (sharded, in_names, out_names, out_avals, mesh)
    return _CACHE[key]


def _run(in_maps, loop_n=1):
    """Execute on 8 cores; returns list of per-core output dicts."""
    import numpy as _np

    sharded, in_names, out_names, out_avals, mesh = _get_exec(loop_n)
    concat_in = [
        _np.concatenate([m[name] for m in in_maps], axis=0) for name in in_names
    ]
    concat_zeros = [
        _np.zeros((NCORES * a.shape[0], *a.shape[1:]), a.dtype) for a in out_avals
    ]
    out_arrs = sharded(*concat_in, *concat_zeros)
    return [
        {
            name: _np.asarray(out_arrs[i]).reshape(NCORES, *out_avals[i].shape)[c]
            for i, name in enumerate(out_names)
        }
        for c in range(NCORES)
    ]


def bench(in_maps, iters=5, loop_n=1):
    """Time device execution with device-resident inputs (excludes host
    transfer of the big operands; zero output buffers are pre-staged)."""
    import time

    import jax
    import numpy as _np
    from jax.sharding import NamedSharding, PartitionSpec

    sharded, in_names, out_names, out_avals, mesh = _get_exec(loop_n)
    sh = NamedSharding(mesh, PartitionSpec("core"))
    concat_in = [
        jax.device_put(
            _np.concatenate([m[name] for m in in_maps], axis=0), sh
        )
        for name in in_names
    ]
    zeros_pool = [
        [
            jax.device_put(
                _np.zeros((NCORES * a.shape[0], *a.shape[1:]), a.dtype), sh
            )
            for a in out_avals
        ]
        for _ in range(iters + 1)
    ]
    for z in zeros_pool:
        for a in z:
            a.block_until_ready()
    # warm-up
    outs = sharded(*concat_in, *zeros_pool[0])
    jax.block_until_ready(outs)
    times = []
    for it in range(iters):
        t0 = time.perf_counter()
        outs = sharded(*concat_in, *zeros_pool[it + 1])
        jax.block_until_ready(outs)
        times.append(time.perf_counter() - t0)
    return times, outs


def kernel(Q_input, K_input, V_input, WQw, WQb, WKw, WKb, WVw, WVb, WOw, WOb,
           _return_results=False):
    import ml_dtypes

    BF = ml_dtypes.bfloat16
    F8 = ml_dtypes.float8_e4m3

    qt = np.ascontiguousarray(np.asarray(Q_input, np.float32).T).astype(F8)
    kt = np.ascontiguousarray(np.asarray(K_input, np.float32).T).astype(F8)
    vt = np.ascontiguousarray(np.asarray(V_input, np.float32).T).astype(BF)

    # triangular keep-mask M[p, c] = 1 if c >= p, and per-chunk skip counts
    tm = (np.arange(128)[None, :] >= np.arange(128)[:, None]).astype(np.float32)
    ct = np.broadcast_to(
        (128.0 * np.arange(16, dtype=np.float32))[None, :], (128, 16)
    ).copy()

    in_maps = []
    for c in range(NCORES):
        h0 = 2 * c
        def _prep_w(w):
            # [2, 64, D] -> [D, 128] -> partition-major [128, 8, 128]
            wt = np.asarray(w, np.float32).transpose(2, 0, 1).reshape(D, 128)
            return np.ascontiguousarray(
                wt.reshape(8, 128, 128).transpose(1, 0, 2)
            ).astype(BF)

        wq = _prep_w(WQw[h0:h0 + 2])
        wk = _prep_w(WKw[h0:h0 + 2])
        wv = _prep_w(WVw[h0:h0 + 2])
        wo = np.ascontiguousarray(
            np.asarray(WOw, np.float32)[:, 128 * c:128 * (c + 1)].T
        ).astype(BF)
        wbf1 = np.concatenate(
            [wq.reshape(128, 1024), wk.reshape(128, 1024)], axis=1
        )
        wbf2 = np.concatenate([wv.reshape(128, 1024), wo], axis=1)
        cf = np.zeros((128, 658), np.float32)
        cf[:, 0] = np.asarray(WQb[h0:h0 + 2], np.float32).reshape(128)
        cf[:, 1] = np.asarray(WKb[h0:h0 + 2], np.float32).reshape(128)
        bv = np.asarray(WVb[h0:h0 + 2], np.float32).reshape(128)
        cf[0, 2:514] = np.tile(bv, 4)
        cf[:, 514:642] = tm
        cf[:, 642:658] = ct
        in_maps.append({
            "qt": qt, "kt": kt, "vt": vt,
            "wbf1": wbf1, "wbf2": wbf2, "cf": np.ascontiguousarray(cf),
        })

    results = _run(in_maps)
    out = np.zeros((D, S), np.float32)
    wc = np.zeros((D,), np.float32)
    for c in range(NCORES):
        out += results[c]["out"].astype(np.float32)
        # wcorr[p, j] is the (negated) correction for output dim d = 128j + p
        wc += results[c]["wcorr"].T.reshape(D)
    out += wc[:, None]
    out = out.T + np.asarray(WOb, np.float32)[None, :]
    if _return_results:
        return out.astype(np.float32), (results, in_maps)
    return out.astype(np.float32)
